# revision 16
# baseline (speedup 1.0000x reference)
"""KGATConv GNN message-passing kernel for 8 Trainium2 NeuronCores.

Strategy (dst-node sharding + on-device AllGather; wire- and host-optimized):
  - Core k owns dst nodes [k*12500, (k+1)*12500) and receives ONLY the edges
    whose dst it owns, so each core's segment-sum over its 98 dst windows is
    COMPLETE locally -- no cross-core reduction of partials.
  - Each core ships only its own nfeat shard (int8 row-quantized, 1.6MB); an
    on-device AllGather (fast NeuronLink, not the slow axon tunnel) builds the
    full [100352,128] int8 node table every core gathers src rows from.
  - Edge tables ship as 4 uint8 planes of one uint32 pack
    ((src_row<<7 | dst_off) << 8 | w_u8); the device unpacks with shifts/ands.
    u8 edge weights stay EXACT integers on device -- the dequant step is
    folded into the per-row activation scale.  Edges bucket by dst window
    only (98 windows, ~2040 edges each), so chunk-of-128 padding waste is ~6%.
  - Device, per own window t: per chunk, indirect-DMA gather of 128 int8 rows
    (widened to f16 on ACT); DVE builds A[p,j] = w_p * (dstoff_p==j); PE
    matmul-accumulates h_nb = A^T @ g in PSUM f32.  Finalize inline: X =
    nfeat_own * h_nb, X^T via PE transpose, out = X @ W^T, LeakyReLU on ACT.
  - Output ships as ASYMMETRIC 6-bit (leaky_relu negatives are 100x smaller
    than positives, so positives get levels 0..55 / per-row posmax and
    negatives 0..8 / per-row negmax), packed 4 values per 3 bytes: 9.6MB
    instead of 12.8MB over the ~30MB/s D2H tunnel.
  - Host preprocessing (single-CPU box!) runs through a small C helper
    compiled at import (numpy fallback): nfeat quantization, edge bucketing
    (two passes, no argsort), and 6-bit output decode.
  - Runner avoids run_bass_kernel_spmd's donated host zeros (13MB of zeros
    over the tunnel per call): output-named operands are cached
    device-resident buffers, reused non-donated (every output element is
    written by the kernel, so their content is irrelevant).
  - nfeat H2D starts as one async sharded put ~40ms into the call and
    overlaps the edge bucketing; edge planes stage inside the jit call; the
    single packed output (bytes + embedded per-row f16 scales) is fetched
    per-shard in parallel and decoded while later shards are in flight.
"""

import sys

sys.path.insert(0, "/opt/trn_rl_repo")

import ctypes
import hashlib
import os
import subprocess
import tempfile
from concurrent.futures import ThreadPoolExecutor
from contextlib import ExitStack

import numpy as np
import jax
import jax.numpy as jnp

for _k, _v in (
    ("jax_compilation_cache_dir", "/tmp/jax_pcc"),
    ("jax_persistent_cache_min_compile_time_secs", 0),
    ("jax_persistent_cache_min_entry_size_bytes", 0),
):
    try:
        jax.config.update(_k, _v)
    except Exception:
        pass

from jax.sharding import Mesh, NamedSharding, PartitionSpec
from jax.experimental.shard_map import shard_map

import concourse.bass as bass
import concourse.mybir as mybir
import concourse.tile as tile
import concourse.bass2jax as b2j

N_CORES = 8
D = 128
WIN = 128
NPC = 12500  # nodes owned per core
PAD = 12544  # NPC rounded up to a whole number of 128-row windows
NW_OWN = PAD // WIN  # 98 dst windows per core
NROWS = N_CORES * PAD  # rows in the AllGathered node table
KPOS = 55.0  # 6-bit positive levels
KNEG = 8.0  # 6-bit negative levels

_entry_cache = {}
_pool = ThreadPoolExecutor(max_workers=N_CORES)
_mesh_cache = {}

# ---------------------------------------------------------------------------
# C helper (single-core host: numpy's many passes over 1.6M edges / 12.8M
# floats dominate the wall; these fused loops are ~10x cheaper)
# ---------------------------------------------------------------------------
_C_SRC = r"""
#include <stdint.h>
#include <math.h>

void quant_all(const float* nf, int64_t n, int64_t npc, int64_t pad,
               int8_t* out, float* scale) {
    for (int64_t i = 0; i < n; i++) {
        const float* row = nf + i * 128;
        float mx = 0.f;
        for (int j = 0; j < 128; j++) {
            float a = fabsf(row[j]);
            if (a > mx) mx = a;
        }
        float s = (mx > 1.27e-10f ? mx : 1.27e-10f) * (1.f / 127.f);
        scale[i] = s;
        float inv = 1.f / s;
        int8_t* orow = out + ((i / npc) * pad + (i % npc)) * 128;
        for (int j = 0; j < 128; j++)
            orow[j] = (int8_t)lrintf(row[j] * inv);
    }
}

void edge_pass1(const int32_t* src, const int32_t* dst, const float* w,
                const float* scale, int64_t E, int32_t* cnt, float* wp,
                float* maxwp) {
    float mx = 1e-30f;
    for (int64_t e = 0; e < E; e++) {
        int32_t d = dst[e];
        int32_t k = d / 12500;
        int32_t r = d - k * 12500;
        cnt[k * 98 + (r >> 7)]++;
        float v = w[e] * scale[src[e]];
        wp[e] = v;
        if (v > mx) mx = v;
    }
    *maxwp = mx;
}

void edge_pass2(const int32_t* src, const int32_t* dst, const float* wp,
                float invstep, int64_t E, const int32_t* col0, int32_t* cur,
                int64_t ct, uint32_t* arr) {
    for (int64_t e = 0; e < E; e++) {
        int32_t d = dst[e];
        int32_t k = d / 12500;
        int32_t r = d - k * 12500;
        int32_t t = r >> 7;
        int32_t off = r & 127;
        int32_t s = src[e];
        uint32_t grow = (uint32_t)(s + (s / 12500) * 44);
        int32_t rank = cur[k * 98 + t]++;
        int64_t col = col0[t] + (rank >> 7);
        int64_t row = rank & 127;
        long wq = lrintf(wp[e] * invstep);
        if (wq > 255) wq = 255;
        if (wq < 0) wq = 0;
        arr[((int64_t)(k * 128 + row)) * ct + col] =
            (((grow << 7) | (uint32_t)off) << 8) | (uint32_t)wq;
    }
}

void deinterleave4(const uint8_t* a, int64_t n, uint8_t* b0, uint8_t* b1,
                   uint8_t* b2, uint8_t* b3) {
    for (int64_t i = 0; i < n; i++) {
        b0[i] = a[4 * i];
        b1[i] = a[4 * i + 1];
        b2[i] = a[4 * i + 2];
        b3[i] = a[4 * i + 3];
    }
}

void decode_shard(const uint8_t* b, const float* PN, int64_t nrows,
                  float* out) {
    for (int64_t i = 0; i < nrows; i++) {
        float sp = PN[2 * i] * (1.f / 55.f);
        float sn = PN[2 * i + 1] * (1.f / 8.f);
        const uint8_t* br = b + i * 100;
        float* orow = out + i * 128;
        for (int g = 0; g < 32; g++) {
            uint32_t v = (uint32_t)br[g] | ((uint32_t)br[32 + g] << 8) |
                         ((uint32_t)br[64 + g] << 16);
            for (int l = 0; l < 4; l++) {
                int32_t q = (int32_t)((v >> (6 * l)) & 63) - 8;
                orow[32 * l + g] = q >= 0 ? q * sp : q * sn;
            }
        }
    }
}
"""


def _load_chelper():
    try:
        h = hashlib.sha256(_C_SRC.encode()).hexdigest()[:16]
        so_path = os.path.join(tempfile.gettempdir(), f"kgat_helper_{h}.so")
        if not os.path.exists(so_path):
            c_path = so_path[:-3] + ".c"
            with open(c_path, "w") as f:
                f.write(_C_SRC)
            subprocess.run(
                ["gcc", "-O3", "-march=native", "-shared", "-fPIC",
                 c_path, "-o", so_path + ".tmp", "-lm"],
                check=True, capture_output=True,
            )
            os.replace(so_path + ".tmp", so_path)
        lib = ctypes.CDLL(so_path)
        i64 = ctypes.c_int64
        pf = ctypes.POINTER(ctypes.c_float)
        pi8 = ctypes.POINTER(ctypes.c_int8)
        pu8 = ctypes.POINTER(ctypes.c_uint8)
        pi32 = ctypes.POINTER(ctypes.c_int32)
        pu32 = ctypes.POINTER(ctypes.c_uint32)
        lib.quant_all.argtypes = [pf, i64, i64, i64, pi8, pf]
        lib.edge_pass1.argtypes = [pi32, pi32, pf, pf, i64, pi32, pf, pf]
        lib.edge_pass2.argtypes = [
            pi32, pi32, pf, ctypes.c_float, i64, pi32, pi32, i64, pu32]
        lib.deinterleave4.argtypes = [pu8, i64, pu8, pu8, pu8, pu8]
        lib.decode_shard.argtypes = [pu8, pf, i64, pf]
        return lib
    except Exception:
        return None


_CLIB = _load_chelper()


def _ptr(a, ctype):
    return a.ctypes.data_as(ctypes.POINTER(ctype))


def _mesh():
    if "m" not in _mesh_cache:
        devices = jax.devices()[:N_CORES]
        mesh = Mesh(np.asarray(devices), ("core",))
        _mesh_cache["m"] = mesh
        _mesh_cache["sh"] = NamedSharding(mesh, PartitionSpec("core"))
    return _mesh_cache["m"], _mesh_cache["sh"]


def _split_excess_waits(nc, maxw=1):
    # This walrus build rejects instructions carrying more than one sync
    # wait.  Move extras onto the immediately preceding instruction of the
    # same engine+queue when it has a free wait slot (engine queues are
    # in-order, so hoisting a monotonic-semaphore wait one slot earlier is
    # equivalent to the NoOp the fallback inserts); otherwise insert NoOps.
    def qkey(i):
        return (i.engine, getattr(i, "queue", None))

    for f in nc.m.functions:
        for bb in f.blocks:
            out = []
            for inst in bb.instructions:
                si = inst.sync_info
                waits = list(si.on_wait) if si and si.on_wait else []
                if len(waits) > maxw:
                    extra, keep = waits[:-maxw], waits[-maxw:]
                    k = len(out) - 1
                    while extra and k >= 0 and qkey(out[k]) == qkey(inst):
                        psi = out[k].sync_info
                        pw = list(psi.on_wait) if psi and psi.on_wait else []
                        room = maxw - len(pw)
                        if room <= 0:
                            break
                        take, extra = extra[-room:], extra[:-room]
                        if psi is None:
                            out[k].sync_info = type(si)(
                                on_wait=list(take), on_update=[]
                            )
                        else:
                            psi.on_wait = pw + list(take)
                        k -= 1
                    for i in range(0, len(extra), maxw):
                        nop = mybir.InstNoOp(
                            name=nc.get_next_instruction_name(), ins=[], outs=[]
                        )
                        nop.engine = inst.engine
                        nop.sync_info = type(si)(
                            on_wait=extra[i : i + maxw], on_update=[]
                        )
                        nc.register_instruction(nop, overwrite=True)
                        out.append(nop)
                    si.on_wait = keep
                out.append(inst)
            bb.instructions[:] = out


def _build_nc(ct, c_list):
    f32 = mybir.dt.float32
    f16 = mybir.dt.float16
    i32 = mybir.dt.int32
    u8 = mybir.dt.uint8
    i8 = mybir.dt.int8
    nc = bass.Bass(num_devices=N_CORES)
    nfeat_d = nc.declare_dram_parameter("nfeat", [PAD, D], i8, isOutput=False)
    scale_d = nc.declare_dram_parameter("scl", [128, NW_OWN], f32, isOutput=False)
    p0_d = nc.declare_dram_parameter("p0", [128, ct], u8, isOutput=False)
    p1_d = nc.declare_dram_parameter("p1", [128, ct], u8, isOutput=False)
    p2_d = nc.declare_dram_parameter("p2", [128, ct], u8, isOutput=False)
    w_d = nc.declare_dram_parameter("wf", [128, ct], u8, isOutput=False)
    wt_d = nc.declare_dram_parameter("wt", [D, D], f32, isOutput=False)
    out_d = nc.declare_dram_parameter("out", [PAD, 100], u8, isOutput=True)

    with tile.TileContext(nc) as tc, ExitStack() as ctx:
        const = ctx.enter_context(tc.tile_pool(name="const", bufs=1))
        up = ctx.enter_context(tc.tile_pool(name="up", bufs=2))
        gp = ctx.enter_context(tc.tile_pool(name="gp", bufs=10))
        ap = ctx.enter_context(tc.tile_pool(name="ap", bufs=4))
        wk = ctx.enter_context(tc.tile_pool(name="wk", bufs=6))
        ps = ctx.enter_context(tc.tile_pool(name="ps", bufs=2, space="PSUM"))
        dram = ctx.enter_context(tc.tile_pool(name="dram", bufs=1, space="DRAM"))

        # ---- AllGather the int8 node table (collectives can't read IO
        # tensors, so bounce the shard through an internal DRAM tile) ----
        nfloc = dram.tile([PAD, D], i8)
        nc.sync.dma_start(out=nfloc[:], in_=nfeat_d[:])
        table = dram.tile([NROWS, D], i8)
        nc.gpsimd.collective_compute(
            "AllGather",
            mybir.AluOpType.bypass,
            replica_groups=[list(range(N_CORES))],
            ins=[nfloc[:].opt()],
            outs=[table[:].opt()],
        )

        # ---- load + unpack edge tables ----
        p0_sb = const.tile([128, ct], u8)
        nc.sync.dma_start(out=p0_sb[:], in_=p0_d[:])
        p1_sb = const.tile([128, ct], u8)
        nc.sync.dma_start(out=p1_sb[:], in_=p1_d[:])
        p2_sb = const.tile([128, ct], u8)
        nc.sync.dma_start(out=p2_sb[:], in_=p2_d[:])
        w8 = const.tile([128, ct], u8)
        nc.sync.dma_start(out=w8[:], in_=w_d[:])
        scale_sb = const.tile([128, NW_OWN], f32)
        nc.sync.dma_start(out=scale_sb[:], in_=scale_d[:])
        wt_sb = const.tile([D, D], f32)
        nc.sync.dma_start(out=wt_sb[:], in_=wt_d[:])

        # A24 = p0 + (p1<<8) + (p2<<16); src row = A24>>7; dst off = A24&127
        p0i = up.tile([128, ct], i32, tag="p0i")
        nc.scalar.copy(out=p0i[:], in_=p0_sb[:])
        p1i = up.tile([128, ct], i32, tag="p1i")
        nc.scalar.copy(out=p1i[:], in_=p1_sb[:])
        p2i = up.tile([128, ct], i32, tag="p2i")
        nc.scalar.copy(out=p2i[:], in_=p2_sb[:])
        nc.vector.tensor_scalar(
            p1i[:], p1i[:], 8, None, mybir.AluOpType.arith_shift_left
        )
        nc.vector.tensor_scalar(
            p2i[:], p2i[:], 16, None, mybir.AluOpType.arith_shift_left
        )
        a24 = up.tile([128, ct], i32, tag="a24")
        nc.vector.tensor_tensor(
            out=a24[:], in0=p0i[:], in1=p1i[:], op=mybir.AluOpType.add
        )
        nc.vector.tensor_tensor(
            out=a24[:], in0=a24[:], in1=p2i[:], op=mybir.AluOpType.add
        )
        src_sb = const.tile([128, ct], i32)
        nc.vector.tensor_scalar(
            src_sb[:], a24[:], 7, None, mybir.AluOpType.logical_shift_right
        )
        offi = up.tile([128, ct], i32, tag="offi")
        nc.vector.tensor_scalar(
            offi[:], a24[:], 127, None, mybir.AluOpType.bitwise_and
        )
        off_sb = const.tile([128, ct], f32)
        nc.scalar.copy(out=off_sb[:], in_=offi[:])
        # u8 edge weights stay exact integers (0..255) here; the dequant step
        # is folded into the per-row activation scale host-side
        w_sb = const.tile([128, ct], f32)
        nc.scalar.copy(out=w_sb[:], in_=w8[:])

        # iota row [p,j]=j and identity [p,j]=(p==j)
        iota_i = const.tile([128, WIN], i32)
        nc.gpsimd.iota(iota_i[:], pattern=[[1, WIN]], base=0, channel_multiplier=0)
        iota_sb = const.tile([128, WIN], f32)
        nc.scalar.copy(out=iota_sb[:], in_=iota_i[:])
        part_i = const.tile([128, WIN], i32)
        nc.gpsimd.iota(part_i[:], pattern=[[0, WIN]], base=0, channel_multiplier=1)
        part_f = const.tile([128, WIN], f32)
        nc.scalar.copy(out=part_f[:], in_=part_i[:])
        ident_sb = const.tile([128, 128], f32)
        nc.vector.tensor_tensor(
            out=ident_sb[:], in0=iota_sb[:], in1=part_f[:], op=mybir.AluOpType.is_equal
        )

        # ---- fused segment-sum + bi-interaction per own dst window ----
        start = 0
        for t in range(NW_OWN):
            c = c_list[t]
            acc = ps.tile([WIN, D], f32, tag="acc")
            for j in range(c):
                col = start + j
                g8 = gp.tile([128, D], i8, tag="g8")
                nc.gpsimd.indirect_dma_start(
                    out=g8[:],
                    out_offset=None,
                    in_=table[:],
                    in_offset=bass.IndirectOffsetOnAxis(
                        ap=src_sb[:, col : col + 1], axis=0
                    ),
                )
                g16 = gp.tile([128, D], f16, tag="g16")
                nc.scalar.copy(out=g16[:], in_=g8[:])
                a_t = ap.tile([128, WIN], f16, tag="A")
                nc.vector.tensor_scalar(
                    a_t[:],
                    iota_sb[:],
                    off_sb[:, col : col + 1],
                    w_sb[:, col : col + 1],
                    mybir.AluOpType.is_equal,
                    mybir.AluOpType.mult,
                )
                nc.tensor.matmul(
                    out=acc[:],
                    lhsT=a_t[:],
                    rhs=g16[:],
                    start=(j == 0),
                    stop=(j == c - 1),
                )
            start += c

            nf8 = wk.tile([WIN, D], i8, tag="nf8")
            nc.sync.dma_start(out=nf8[:], in_=nfeat_d[t * WIN : (t + 1) * WIN, :])
            nf = wk.tile([WIN, D], f32, tag="nf")
            nc.scalar.copy(out=nf[:], in_=nf8[:])
            hb = wk.tile([WIN, D], f32, tag="hb")
            nc.scalar.copy(out=hb[:], in_=acc[:])
            x = wk.tile([WIN, D], f32, tag="x")
            nc.vector.tensor_tensor(
                out=x[:], in0=nf[:], in1=hb[:], op=mybir.AluOpType.mult
            )
            xt_ps = ps.tile([D, WIN], f32, tag="xt")
            nc.tensor.transpose(out=xt_ps[:], in_=x[:], identity=ident_sb[:])
            xt = wk.tile([D, WIN], f32, tag="xts")
            nc.scalar.copy(out=xt[:], in_=xt_ps[:])
            op_ps = ps.tile([WIN, D], f32, tag="op")
            nc.tensor.matmul(
                out=op_ps[:], lhsT=xt[:], rhs=wt_sb[:], start=True, stop=True
            )
            ob32 = wk.tile([WIN, D], f32, tag="ob32")
            # fold the per-row int8 dequant scale (and the u8 edge-weight
            # step) in here: for s>0, lrelu(s*y) == s*lrelu(y)
            nc.scalar.activation(
                out=ob32[:],
                in_=op_ps[:],
                func=mybir.ActivationFunctionType.Lrelu,
                scale=scale_sb[:, t : t + 1],
                alpha=0.01,
            )

            # asymmetric 6-bit wire format: positives quantize to 0..55 of
            # rowmax(relu(y)), negatives to 0..8 of rowmax(relu(-y))
            rp = wk.tile([WIN, D], f32, tag="rp")
            nc.scalar.activation(
                out=rp[:], in_=ob32[:], func=mybir.ActivationFunctionType.Relu
            )
            rn = wk.tile([WIN, D], f32, tag="rn")
            nc.scalar.activation(
                out=rn[:], in_=ob32[:],
                func=mybir.ActivationFunctionType.Relu, scale=-1.0,
            )
            pm = wk.tile([WIN, 1], f32, tag="pm")
            nc.vector.tensor_reduce(
                out=pm[:], in_=rp[:], axis=mybir.AxisListType.X,
                op=mybir.AluOpType.max,
            )
            nm = wk.tile([WIN, 1], f32, tag="nm")
            nc.vector.tensor_reduce(
                out=nm[:], in_=rn[:], axis=mybir.AxisListType.X,
                op=mybir.AluOpType.max,
            )
            pg = wk.tile([WIN, 1], f32, tag="pg")
            nc.vector.tensor_scalar(pg[:], pm[:], 1e-30, None, mybir.AluOpType.max)
            ng = wk.tile([WIN, 1], f32, tag="ng")
            nc.vector.tensor_scalar(ng[:], nm[:], 1e-30, None, mybir.AluOpType.max)
            invp = wk.tile([WIN, 1], f32, tag="invp")
            nc.vector.reciprocal(out=invp[:], in_=pg[:])
            invn = wk.tile([WIN, 1], f32, tag="invn")
            nc.vector.reciprocal(out=invn[:], in_=ng[:])
            qp = wk.tile([WIN, D], f32, tag="qp")
            nc.vector.tensor_scalar(
                qp[:], rp[:], invp[:, 0:1], KPOS,
                mybir.AluOpType.mult, mybir.AluOpType.mult,
            )
            qn = wk.tile([WIN, D], f32, tag="qn")
            nc.vector.tensor_scalar(
                qn[:], rn[:], invn[:, 0:1], KNEG,
                mybir.AluOpType.mult, mybir.AluOpType.mult,
            )
            qf = wk.tile([WIN, D], f32, tag="qf")
            nc.vector.tensor_tensor(
                out=qf[:], in0=qp[:], in1=qn[:], op=mybir.AluOpType.subtract
            )
            # bias to [0,63] and clamp; round-to-nearest at the i32 convert
            qi = wk.tile([WIN, D], i32, tag="qi")
            nc.vector.tensor_scalar(
                qi[:], qf[:], 8.0, 63.0, mybir.AluOpType.add, mybir.AluOpType.min
            )
            # pack 4x6b into 24b: v = qA + qB<<6 + qC<<12 + qD<<18 (quarters)
            v = wk.tile([WIN, 32], i32, tag="v")
            tmp = wk.tile([WIN, 32], i32, tag="tmp")
            nc.vector.tensor_scalar(
                tmp[:], qi[:, 32:64], 6, None, mybir.AluOpType.arith_shift_left
            )
            nc.vector.tensor_tensor(
                out=v[:], in0=qi[:, 0:32], in1=tmp[:], op=mybir.AluOpType.add
            )
            nc.vector.tensor_scalar(
                tmp[:], qi[:, 64:96], 12, None, mybir.AluOpType.arith_shift_left
            )
            nc.vector.tensor_tensor(
                out=v[:], in0=v[:], in1=tmp[:], op=mybir.AluOpType.add
            )
            nc.vector.tensor_scalar(
                tmp[:], qi[:, 96:128], 18, None, mybir.AluOpType.arith_shift_left
            )
            nc.vector.tensor_tensor(
                out=v[:], in0=v[:], in1=tmp[:], op=mybir.AluOpType.add
            )
            # bit ops can't change dtype; extract bytes in i32 then convert.
            # cols 96:100 carry the per-row f16 pos/neg scales, written via a
            # separate small DMA (f16-bitcast SBUF writes into the u8 tile
            # measured ~0.4ms EACH on ACT -- a DMA with a DRAM-side bitcast
            # view is free)
            obu = wk.tile([WIN, 96], u8, tag="obu")
            tmp2 = wk.tile([WIN, 32], i32, tag="tmp2")
            nc.vector.tensor_scalar(
                tmp2[:], v[:], 255, None, mybir.AluOpType.bitwise_and
            )
            nc.scalar.copy(out=obu[:, 0:32], in_=tmp2[:])
            nc.vector.tensor_scalar(
                tmp[:], v[:], 8, None, mybir.AluOpType.logical_shift_right
            )
            nc.vector.tensor_scalar(
                tmp2[:], tmp[:], 255, None, mybir.AluOpType.bitwise_and
            )
            nc.scalar.copy(out=obu[:, 32:64], in_=tmp2[:])
            nc.vector.tensor_scalar(
                tmp2[:], v[:], 16, None, mybir.AluOpType.logical_shift_right
            )
            nc.scalar.copy(out=obu[:, 64:96], in_=tmp2[:])
            pn16 = wk.tile([WIN, 2], f16, tag="pn16")
            nc.scalar.copy(out=pn16[:, 0:1], in_=pg[:])
            nc.scalar.copy(out=pn16[:, 1:2], in_=ng[:])
            nc.sync.dma_start(out=out_d[t * WIN : (t + 1) * WIN, 0:96], in_=obu[:])
            nc.sync.dma_start(
                out=out_d[t * WIN : (t + 1) * WIN, 96:100].bitcast(f16),
                in_=pn16[:],
            )
    _split_excess_waits(nc)
    return nc


def _get_entry(ct, c_list):
    key = (ct, tuple(c_list))
    if key in _entry_cache:
        return _entry_cache[key]

    nc = _build_nc(ct, c_list)
    jb = nc.to_json_bytes()
    nc.to_json_bytes = lambda: jb

    b2j.install_neuronx_cc_hook()
    partition_name = nc.partition_id_tensor.name if nc.partition_id_tensor else None
    in_names, out_names, out_avals = [], [], []
    for alloc in nc.m.functions[0].allocations:
        if not isinstance(alloc, mybir.MemoryLocationSet):
            continue
        name = alloc.memorylocations[0].name
        if alloc.kind == "ExternalInput":
            if name != partition_name:
                in_names.append(name)
        elif alloc.kind == "ExternalOutput":
            out_names.append(name)
            out_avals.append(
                jax.core.ShapedArray(
                    tuple(alloc.tensor_shape), mybir.dt.np(alloc.dtype)
                )
            )
    n_params = len(in_names)
    all_names = list(in_names) + out_names
    if partition_name is not None:
        all_names.append(partition_name)

    def _body(*args):
        operands = list(args)
        if partition_name is not None:
            operands.append(b2j.partition_id_tensor())
        return tuple(
            b2j._bass_exec_p.bind(
                *operands,
                out_avals=tuple(out_avals),
                in_names=tuple(all_names),
                out_names=tuple(out_names),
                lowering_input_output_aliases=(),
                sim_require_finite=True,
                sim_require_nnan=True,
                nc=nc,
            )
        )

    mesh, sh = _mesh()
    n_outs = len(out_avals)
    sharded = jax.jit(
        shard_map(
            _body,
            mesh=mesh,
            in_specs=(PartitionSpec("core"),) * (n_params + n_outs),
            out_specs=(PartitionSpec("core"),) * n_outs,
            check_rep=False,
        )
    )
    # Output-named operands: the NEFF writes every element of both outputs,
    # so these buffers only need the right shape.  Create them ON DEVICE once
    # and reuse non-donated -- never ship 13MB of zeros over the tunnel.
    zmake = jax.jit(
        lambda: tuple(
            jnp.zeros((N_CORES * a.shape[0], *a.shape[1:]), a.dtype)
            for a in out_avals
        ),
        out_shardings=tuple(sh for _ in out_avals),
    )
    zbufs = zmake()
    entry = {
        "sharded": sharded,
        "in_names": in_names,
        "out_names": out_names,
        "zbufs": zbufs,
    }
    _entry_cache[key] = entry
    return entry


def _prep_numpy(src, dst, w, scale):
    """numpy fallback for the C edge passes; returns (cnt, wp, maxwp)."""
    kd = dst // NPC
    r = dst - kd * NPC
    key = kd * NW_OWN + (r >> 7)
    cnt = np.bincount(key, minlength=N_CORES * NW_OWN).astype(np.int32)
    wp = w * scale[src]
    return key, r, cnt, wp, float(wp.max())


def _kernel_impl(nfeat, edge_src, edge_dst, edge_w, W, npc=NPC, trace=False):
    n, d = nfeat.shape
    assert d == D and npc == NPC and npc * N_CORES == n
    E = edge_src.shape[0]

    src = np.ascontiguousarray(edge_src, dtype=np.int32)
    dst = np.ascontiguousarray(edge_dst, dtype=np.int32)
    w = np.ascontiguousarray(edge_w, dtype=np.float32)
    nfeat = np.ascontiguousarray(nfeat, dtype=np.float32)

    mesh, sh = _mesh()
    cf = ctypes.c_float

    # ---- quantize nfeat (int8 per-row) and start its async H2D put ----
    nfeat_pad = np.zeros((N_CORES * PAD, D), np.int8)
    scale = np.empty(n, np.float32)
    if _CLIB is not None:
        _CLIB.quant_all(
            _ptr(nfeat, cf), n, NPC, PAD, _ptr(nfeat_pad, ctypes.c_int8),
            _ptr(scale, cf),
        )
    else:
        absmax = np.maximum(nfeat.max(axis=1), -nfeat.min(axis=1))
        np.multiply(np.maximum(absmax, 1.27e-10), 1.0 / 127.0, out=scale)
        q8f = nfeat * (1.0 / scale)[:, None]
        np.rint(q8f, out=q8f)
        np.copyto(
            nfeat_pad.reshape(N_CORES, PAD, D)[:, :NPC],
            q8f.reshape(N_CORES, NPC, D), casting="unsafe",
        )
    nfeat_dev = jax.device_put(nfeat_pad, sh)  # async; wire starts now

    # ---- bucket edges by dst owner + own dst window (two C passes) ----
    if _CLIB is not None:
        cnt = np.zeros(N_CORES * NW_OWN, np.int32)
        wp = np.empty(E, np.float32)
        mw = np.zeros(1, np.float32)
        _CLIB.edge_pass1(
            _ptr(src, ctypes.c_int32), _ptr(dst, ctypes.c_int32), _ptr(w, cf),
            _ptr(scale, cf), E, _ptr(cnt, ctypes.c_int32), _ptr(wp, cf),
            _ptr(mw, cf),
        )
        maxwp = float(mw[0])
    else:
        key, r, cnt, wp, maxwp = _prep_numpy(src, dst, w, scale)

    cnt2 = cnt.reshape(N_CORES, NW_OWN)
    c_arr = np.maximum(1, -(-cnt2 // 128)).max(axis=0).astype(np.int32)  # [98]
    c_list = [int(v) for v in c_arr]
    ct = int(c_arr.sum())
    col0 = np.ascontiguousarray(
        np.concatenate([[0], np.cumsum(c_arr)[:-1]]).astype(np.int32)
    )
    step = max(maxwp, 1e-30) * (1.0 / 255.0)

    arr = np.zeros(N_CORES * 128 * ct, np.uint32)
    if _CLIB is not None:
        cur = np.zeros(N_CORES * NW_OWN, np.int32)
        _CLIB.edge_pass2(
            _ptr(src, ctypes.c_int32), _ptr(dst, ctypes.c_int32), _ptr(wp, cf),
            cf(1.0 / step), E, _ptr(col0, ctypes.c_int32),
            _ptr(cur, ctypes.c_int32), ct, _ptr(arr, ctypes.c_uint32),
        )
    else:
        order = np.argsort(key.astype(np.int16), kind="stable")
        ks = key[order].astype(np.int32)
        bstart = np.concatenate([[0], np.cumsum(cnt)])[:-1].astype(np.int32)
        rank = np.arange(E, dtype=np.int32) - bstart[ks]
        k_s = ks // NW_OWN
        t_s = ks - k_s * NW_OWN
        col = col0[t_s] + (rank >> 7)
        rowi = rank & 127
        flat = (k_s * 128 + rowi) * ct + col
        so = src // NPC
        grow = src + so * (PAD - NPC)
        a24 = (grow << 7) | (r & 127)
        wq = np.minimum(np.rint(wp * (1.0 / step)), 255.0).astype(np.uint32)
        comb = (a24.astype(np.uint32) << 8) | wq
        arr[flat] = comb[order]

    shp = (N_CORES * 128, ct)
    nsl = N_CORES * 128 * ct
    wf_arr = np.empty(nsl, np.uint8)
    p0_arr = np.empty(nsl, np.uint8)
    p1_arr = np.empty(nsl, np.uint8)
    p2_arr = np.empty(nsl, np.uint8)
    if _CLIB is not None:
        _CLIB.deinterleave4(
            _ptr(arr.view(np.uint8), ctypes.c_uint8), nsl,
            _ptr(wf_arr, ctypes.c_uint8), _ptr(p0_arr, ctypes.c_uint8),
            _ptr(p1_arr, ctypes.c_uint8), _ptr(p2_arr, ctypes.c_uint8),
        )
    else:
        bv = arr.view(np.uint8).reshape(-1, 4)
        wf_arr[:] = bv[:, 0]
        p0_arr[:] = bv[:, 1]
        p1_arr[:] = bv[:, 2]
        p2_arr[:] = bv[:, 3]


    # scale [128, NW_OWN] per core: partition p, col t <-> own row t*128+p;
    # folds the u8 edge-weight dequant step in alongside the row scale
    scale_pad = np.zeros((N_CORES, PAD), np.float32)
    scale_pad[:, :NPC] = scale.reshape(N_CORES, NPC) * step
    scale_arr = np.ascontiguousarray(
        scale_pad.reshape(N_CORES, NW_OWN, 128).transpose(0, 2, 1)
    ).reshape(N_CORES * 128, NW_OWN)
    wt = np.ascontiguousarray(np.asarray(W).T.astype(np.float32))
    wt_g = np.ascontiguousarray(
        np.broadcast_to(wt, (N_CORES, D, D)).reshape(N_CORES * D, D)
    )

    entry = _get_entry(ct, c_list)
    arrays = {
        "nfeat": nfeat_dev,
        "scl": scale_arr,
        "p0": p0_arr.reshape(shp),
        "p1": p1_arr.reshape(shp),
        "p2": p2_arr.reshape(shp),
        "wf": wf_arr.reshape(shp),
        "wt": wt_g,
    }
    args = [arrays[name] for name in entry["in_names"]]
    out_arrs = entry["sharded"](*args, *entry["zbufs"])
    o_by_name = dict(zip(entry["out_names"], out_arrs))

    out = np.empty((n, D), np.float32)
    ob_g = o_by_name["out"]

    def _assemble(k):
        bts = np.ascontiguousarray(np.asarray(ob_g.addressable_shards[k].data))
        kk = ob_g.addressable_shards[k].index[0].start // PAD
        dst_blk = out[kk * NPC : (kk + 1) * NPC]
        PN = np.ascontiguousarray(bts[:NPC, 96:100]).view(np.float16).astype(
            np.float32
        )
        if _CLIB is not None:
            _CLIB.decode_shard(
                _ptr(bts, ctypes.c_uint8), _ptr(PN, cf), NPC, _ptr(dst_blk, cf)
            )
        else:
            b0 = bts[:NPC, 0:32].astype(np.int32)
            b1 = bts[:NPC, 32:64].astype(np.int32)
            b2 = bts[:NPC, 64:96].astype(np.int32)
            v = b0 | (b1 << 8) | (b2 << 16)
            sp = PN[:, 0:1] * (1.0 / KPOS)
            sn = PN[:, 1:2] * (1.0 / KNEG)
            for l in range(4):
                q = ((v >> (6 * l)) & 63) - 8
                dst_blk[:, 32 * l : 32 * l + 32] = np.where(q >= 0, q * sp, q * sn)

    list(_pool.map(_assemble, range(N_CORES)))
    return out


def kernel(nfeat, edge_src, edge_dst, edge_w, W):
    return _kernel_impl(
        np.asarray(nfeat),
        np.asarray(edge_src),
        np.asarray(edge_dst),
        np.asarray(edge_w),
        np.asarray(W),
        npc=NPC,
    )
